# revision 6
# baseline (speedup 1.0000x reference)
"""GPT-2 single-token decode on 8 TRN2 NeuronCores (tensor-parallel).

Sharding: 16 virtual heads (12 real + 4 zero-pad), 2 per core; MLP hidden
3072 split 384/core; lm_head vocab split 6400/core (padded 51200).
Per-layer sync: AllGather of partial residual contributions + local
ones-matmul reduce.  Weight GEMVs run on PE with x stationary (f32r);
QK scores + lm_head dots run on DVE (natural [out,in] layouts).
KV cache rows are interleaved per-core as [K0|K1|V0|1|V1|1]; the ones
columns let the PV matmul accumulate the softmax denominator for free.
The new-token k/v never enters the cache tiles - its score/value term is
handled inline on partition 0.  Softmax skips max-subtraction (scores
are O(1) for this model; exp is safe).
"""
import numpy as np
from contextlib import ExitStack

import concourse.bass as bass
import concourse.tile as tile
from concourse import bacc, mybir
from concourse import bass_utils

dt = mybir.dt
F32 = dt.float32
F32R = dt.float32r
AF = mybir.ActivationFunctionType
OP = mybir.AluOpType
AX = mybir.AxisListType

NCORES = 8
L, H, D, E, V, MAXS = 12, 12, 64, 768, 50257, 1024
S = 1023            # past length (static)
NT = 8              # seq tiles of 128
HID = 384           # mlp hidden per core
VPC = 6400          # vocab rows per core (padded); 50 tiles of 128
NLM = VPC // 128
KE = E // 128       # 6 k-tiles over E
KH = HID // 128     # 3 k-tiles over HID
ROW = 258           # kv row: K0|K1|V0|1|V1|1


def build_nc(nlayers=L, nlm=NLM):
    nc = bacc.Bacc("TRN2", target_bir_lowering=False, debug=False,
                   num_devices=NCORES)
    x0_d = nc.dram_tensor("x0p", [128, KE], F32, kind="ExternalInput")
    qkvw_d = nc.dram_tensor("qkvw", [nlayers, E, 384], F32R, kind="ExternalInput")
    fcw_d = nc.dram_tensor("fcw", [nlayers, E, HID], F32R, kind="ExternalInput")
    mlpw_d = nc.dram_tensor("mlpw", [nlayers, HID, E], F32R, kind="ExternalInput")
    projw_d = nc.dram_tensor("projw", [nlayers, 128, E], F32R, kind="ExternalInput")
    bias_d = nc.dram_tensor("bias", [nlayers, 2304], F32, kind="ExternalInput")
    lnp_d = nc.dram_tensor("lnp", [nlayers, 128, 4 * KE], F32, kind="ExternalInput")
    lnf_d = nc.dram_tensor("lnf", [128, 2 * KE], F32, kind="ExternalInput")
    kvc_d = nc.dram_tensor("kvc", [nlayers, MAXS, ROW], F32, kind="ExternalInput")
    wte_d = nc.dram_tensor("wtep", [nlm, 128, E], F32, kind="ExternalInput")
    id_d = nc.dram_tensor("ident", [128, 128], F32, kind="ExternalInput")
    logits_d = nc.dram_tensor("logits", [128, nlm], F32, kind="ExternalOutput")
    kvnew_d = nc.dram_tensor("kvnew", [nlayers, 256], F32, kind="ExternalOutput")

    with tile.TileContext(nc) as tc, ExitStack() as ctx:
        state = ctx.enter_context(tc.tile_pool(name="state", bufs=1))
        wpool = ctx.enter_context(tc.tile_pool(name="wpool", bufs=3))
        kvpool = ctx.enter_context(tc.tile_pool(name="kvpool", bufs=2))
        cpool = ctx.enter_context(tc.tile_pool(name="cpool", bufs=3))
        work = ctx.enter_context(tc.tile_pool(name="work", bufs=2))
        lmw = ctx.enter_context(tc.tile_pool(name="lmw", bufs=4))
        ps = ctx.enter_context(tc.tile_pool(name="ps", bufs=1, space="PSUM"))
        dram = ctx.enter_context(tc.tile_pool(name="dram", bufs=2, space="DRAM"))

        def ps_a(name):   # [1, <=384] rows, 4 shared banks
            return ps.tile([1, 384], F32, tag="psa", bufs=4, name=name)

        def ps_b(name):   # [128, <=384], 2 shared banks
            return ps.tile([128, 384], F32, tag="psb", bufs=2, name=name)

        def ps_c(name):   # [128, <=8], 2 shared banks
            return ps.tile([128, 8], F32, tag="psc", bufs=2, name=name)

        # persistent state
        hP = state.tile([128, KE], F32)
        nc.sync.dma_start(hP[:], x0_d[:])
        ones_col = state.tile([128, 1], F32)
        nc.vector.memset(ones_col[:], 1.0)
        ones_row = state.tile([1, 128], F32)
        nc.vector.memset(ones_row[:], 1.0)
        ones8 = state.tile([NCORES, 1], F32)
        nc.vector.memset(ones8[:], 1.0)
        one1 = state.tile([1, 1], F32)
        nc.vector.memset(one1[:], 1.0)
        ident = state.tile([128, 128], F32)
        nc.sync.dma_start(ident[:], id_d[:])
        epsT = state.tile([1, 1], F32)
        nc.vector.memset(epsT[:], 1e-5)
        sq2piT = state.tile([1, 1], F32)
        nc.vector.memset(sq2piT[:], 0.7978845608028654)
        lnfp = state.tile([128, 2 * KE], F32)
        nc.sync.dma_start(lnfp[:], lnf_d[:])
        logits_sb = state.tile([128, nlm], F32)

        def layernorm(src, g_ap, b_ap, tag):
            """src [128,KE] F32 -> (xn F32, xr F32R), LN over all 768."""
            sq = work.tile([128, KE], F32, tag="sq", name="sq")
            nc.vector.tensor_tensor(out=sq[:], in0=src[:], in1=src[:], op=OP.mult)
            st = ps_a("st")
            nc.tensor.matmul(st[0:1, 0:KE], ones_col[:], src[:],
                             start=True, stop=True)
            nc.tensor.matmul(st[0:1, KE:2 * KE], ones_col[:], sq[:],
                             start=True, stop=True)
            mvr = work.tile([1, 4], F32, tag="mvr", name="mvr")
            nc.vector.tensor_reduce(out=mvr[0:1, 0:1], in_=st[0:1, 0:KE],
                                    axis=AX.X, op=OP.add)
            nc.vector.tensor_reduce(out=mvr[0:1, 3:4], in_=st[0:1, KE:2 * KE],
                                    axis=AX.X, op=OP.add)
            nc.scalar.mul(mvr[0:1, 0:1], mvr[0:1, 0:1], 1.0 / E)
            nc.scalar.activation(mvr[0:1, 1:2], mvr[0:1, 0:1], AF.Square)
            nc.vector.scalar_tensor_tensor(
                out=mvr[0:1, 2:3], in0=mvr[0:1, 3:4], scalar=1.0 / E,
                in1=mvr[0:1, 1:2], op0=OP.mult, op1=OP.subtract)
            nc.scalar.activation(mvr[0:1, 1:2], mvr[0:1, 2:3], AF.Sqrt,
                                 bias=epsT[0:1, 0:1])
            nc.vector.reciprocal(mvr[0:1, 1:2], mvr[0:1, 1:2])
            mb = ps_c("mb")
            nc.tensor.matmul(mb[:, 0:2], ones_row[:], mvr[0:1, 0:2],
                             start=True, stop=True)
            xc = work.tile([128, KE], F32, tag="xc", name="xc")
            nc.vector.tensor_scalar(out=xc[:], in0=src[:], scalar1=mb[:, 0:1],
                                    scalar2=None, op0=OP.subtract)
            xn = work.tile([128, KE], F32, tag=f"xn_{tag}", name=f"xn_{tag}")
            nc.vector.scalar_tensor_tensor(out=xn[:], in0=xc[:], scalar=mb[:, 1:2],
                                           in1=g_ap, op0=OP.mult, op1=OP.mult)
            nc.vector.tensor_tensor(out=xn[:], in0=xn[:], in1=b_ap, op=OP.add)
            xr = work.tile([128, KE], F32R, tag=f"xr_{tag}", name=f"xr_{tag}")
            nc.vector.tensor_copy(xr[:], xn[:])
            return xn, xr

        def sync_residual(partial_sb):
            """AllGather partial [1,E] from all cores, reduce, add into hP."""
            agin = dram.tile([1, E], F32, tag="agin", name="agin")
            agout = dram.tile([NCORES, E], F32, addr_space="Shared",
                              tag="agout", name="agout")
            nc.sync.dma_start(agin[:], partial_sb[:])
            nc.gpsimd.collective_compute(
                "AllGather", OP.bypass,
                replica_groups=[list(range(NCORES))],
                ins=[agin.opt()], outs=[agout.opt()])
            gath = work.tile([NCORES, E], F32, tag="gath", name="gath")
            nc.sync.dma_start(gath[:], agout[:])
            redA = ps_a("redA")
            redB = ps_a("redB")
            nc.tensor.matmul(redA[0:1, :], ones8[:], gath[:, 0:384],
                             start=True, stop=True)
            nc.tensor.matmul(redB[0:1, :], ones8[:], gath[:, 384:768],
                             start=True, stop=True)
            hrow = work.tile([1, E], F32, tag="hrow", name="hrow")
            nc.vector.tensor_copy(hrow[0:1, 0:384], redA[0:1, :])
            nc.vector.tensor_copy(hrow[0:1, 384:768], redB[0:1, :])
            hT = ps_c("hT")
            for k in range(KE):
                nc.tensor.transpose(hT[:, k:k + 1],
                                    hrow[0:1, 128 * k:128 * (k + 1)], one1[:])
            nc.vector.tensor_tensor(out=hP[:], in0=hP[:], in1=hT[:, 0:KE],
                                    op=OP.add)

        for l in range(nlayers):
            # ---- per-layer DMAs ----
            qkvw = wpool.tile([128, KE * 384], F32R, tag="qkvw", name="qkvw")
            nc.sync.dma_start(
                qkvw[:].rearrange("p (k n) -> p k n", k=KE),
                qkvw_d[l].rearrange("(k p) n -> p k n", p=128))
            fcw = wpool.tile([128, KE * HID], F32R, tag="fcw", name="fcw")
            nc.sync.dma_start(
                fcw[:].rearrange("p (k n) -> p k n", k=KE),
                fcw_d[l].rearrange("(k p) n -> p k n", p=128))
            mlpw = wpool.tile([128, KH * E], F32R, tag="mlpw", name="mlpw")
            nc.sync.dma_start(
                mlpw[:].rearrange("p (k n) -> p k n", k=KH),
                mlpw_d[l].rearrange("(k p) n -> p k n", p=128))
            projw = wpool.tile([128, E], F32R, tag="projw", name="projw")
            nc.sync.dma_start(projw[:], projw_d[l])
            bias = cpool.tile([1, 2304], F32, tag="bias", name="bias")
            nc.sync.dma_start(bias[:], bias_d[l:l + 1, :])
            lnp = cpool.tile([128, 4 * KE], F32, tag="lnp", name="lnp")
            nc.sync.dma_start(lnp[:], lnp_d[l])
            kv = kvpool.tile([128, NT * ROW], F32, tag="kv", name="kv")
            nc.sync.dma_start(
                kv[:].rearrange("p (t n) -> p t n", t=NT),
                kvc_d[l].rearrange("(t p) n -> p t n", p=128))

            # ---- LN1 + qkv ----
            xn, xr = layernorm(hP, lnp[:, 0:KE], lnp[:, KE:2 * KE], "a")
            pqkv = ps_a("pqkv")
            for k in range(KE):
                nc.tensor.matmul(pqkv[0:1, :], xr[:, k:k + 1],
                                 qkvw[:, 384 * k:384 * (k + 1)],
                                 start=(k == 0), stop=(k == KE - 1))
            qkv = work.tile([1, 384], F32, tag="qkv", name="qkv")
            nc.vector.tensor_tensor(out=qkv[:], in0=pqkv[0:1, :],
                                    in1=bias[0:1, 0:384], op=OP.add)
            nc.sync.dma_start(kvnew_d[l:l + 1, :], qkv[0:1, 128:384])

            # ---- attention ----
            qb = ps_b("qb")
            nc.tensor.matmul(qb[:, 0:128], ones_row[:], qkv[0:1, 0:128],
                             start=True, stop=True)
            qq = work.tile([128, 128], F32, tag="qq", name="qq")
            nc.vector.tensor_copy(qq[:], qb[:, 0:128])

            scores = work.tile([128, 16], F32, tag="scores", name="scores")
            nc.vector.memset(scores[:], 0.0)
            for t in range(NT):
                P = 127 if t == NT - 1 else 128
                prod = work.tile([128, 128], F32, tag="prod", name="prod")
                nc.vector.tensor_tensor(out=prod[0:P, :],
                                        in0=kv[0:P, ROW * t:ROW * t + 128],
                                        in1=qq[0:P, :], op=OP.mult)
                nc.vector.tensor_reduce(out=scores[0:P, t:t + 1],
                                        in_=prod[0:P, 0:64], axis=AX.X, op=OP.add)
                nc.vector.tensor_reduce(out=scores[0:P, 8 + t:9 + t],
                                        in_=prod[0:P, 64:128], axis=AX.X, op=OP.add)
            pexp = work.tile([128, 16], F32, tag="pexp", name="pexp")
            nc.scalar.activation(pexp[:], scores[:], AF.Exp, scale=0.125)
            # new-token score, partition 0
            pn = work.tile([1, 130], F32, tag="pn", name="pn")
            nc.vector.tensor_tensor(out=pn[0:1, 2:130], in0=qkv[0:1, 0:128],
                                    in1=qkv[0:1, 128:256], op=OP.mult)
            nc.vector.tensor_reduce(out=pn[0:1, 0:1], in_=pn[0:1, 2:66],
                                    axis=AX.X, op=OP.add)
            nc.vector.tensor_reduce(out=pn[0:1, 1:2], in_=pn[0:1, 66:130],
                                    axis=AX.X, op=OP.add)
            nc.scalar.activation(pn[0:1, 0:2], pn[0:1, 0:2], AF.Exp, scale=0.125)

            po = [ps_a("po0"), ps_a("po1")]
            for h in range(2):
                for t in range(NT):
                    P = 127 if t == NT - 1 else 128
                    base = ROW * t + 128 + 65 * h
                    nc.tensor.matmul(po[h][0:1, 0:65],
                                     pexp[0:P, 8 * h + t:8 * h + t + 1],
                                     kv[0:P, base:base + 65],
                                     start=(t == 0), stop=(t == NT - 1))
            onrm = work.tile([1, 128], F32, tag="onrm", name="onrm")
            z2 = work.tile([1, 2], F32, tag="z2", name="z2")
            for h in range(2):
                nc.vector.scalar_tensor_tensor(
                    out=onrm[0:1, 64 * h:64 * (h + 1)],
                    in0=qkv[0:1, 256 + 64 * h:256 + 64 * (h + 1)],
                    scalar=pn[0:1, h:h + 1], in1=po[h][0:1, 0:64],
                    op0=OP.mult, op1=OP.add)
                nc.vector.tensor_tensor(out=z2[0:1, h:h + 1],
                                        in0=po[h][0:1, 64:65],
                                        in1=pn[0:1, h:h + 1], op=OP.add)
            nc.vector.reciprocal(z2[:], z2[:])
            for h in range(2):
                nc.vector.tensor_scalar(out=onrm[0:1, 64 * h:64 * (h + 1)],
                                        in0=onrm[0:1, 64 * h:64 * (h + 1)],
                                        scalar1=z2[0:1, h:h + 1], scalar2=None,
                                        op0=OP.mult)
            poT = ps_c("poT")
            nc.tensor.transpose(poT[:, 0:1], onrm[:], one1[:])
            oT = work.tile([128, 1], F32R, tag="oT", name="oT")
            nc.vector.tensor_copy(oT[:], poT[:, 0:1])
            pprA = ps_a("pprA")
            pprB = ps_a("pprB")
            nc.tensor.matmul(pprA[0:1, :], oT[:], projw[:, 0:384],
                             start=True, stop=True)
            nc.tensor.matmul(pprB[0:1, :], oT[:], projw[:, 384:768],
                             start=True, stop=True)
            pa = work.tile([1, E], F32, tag="pa", name="pa")
            nc.vector.tensor_tensor(out=pa[0:1, 0:384], in0=pprA[0:1, :],
                                    in1=bias[0:1, 384:768], op=OP.add)
            nc.vector.tensor_tensor(out=pa[0:1, 384:768], in0=pprB[0:1, :],
                                    in1=bias[0:1, 768:1152], op=OP.add)
            sync_residual(pa)

            # ---- LN2 + MLP ----
            x2n, x2r = layernorm(hP, lnp[:, 2 * KE:3 * KE], lnp[:, 3 * KE:4 * KE],
                                 "m")
            pfc = ps_a("pfc")
            for k in range(KE):
                nc.tensor.matmul(pfc[0:1, :], x2r[:, k:k + 1],
                                 fcw[:, HID * k:HID * (k + 1)],
                                 start=(k == 0), stop=(k == KE - 1))
            u = work.tile([1, HID], F32, tag="u", name="u")
            nc.vector.tensor_tensor(out=u[:], in0=pfc[0:1, :],
                                    in1=bias[0:1, 1152:1536], op=OP.add)
            # tanh-approx gelu: 0.5*x*(1+tanh(c*(x+0.044715 x^3)))
            g1 = work.tile([1, HID], F32, tag="g1", name="g1")
            nc.vector.tensor_tensor(out=g1[:], in0=u[:], in1=u[:], op=OP.mult)
            nc.vector.tensor_tensor(out=g1[:], in0=g1[:], in1=u[:], op=OP.mult)
            nc.vector.scalar_tensor_tensor(out=g1[:], in0=g1[:], scalar=0.044715,
                                           in1=u[:], op0=OP.mult, op1=OP.add)
            nc.scalar.activation(g1[:], g1[:], AF.Tanh, scale=sq2piT[0:1, 0:1])
            g2 = work.tile([1, HID], F32, tag="g2", name="g2")
            nc.vector.tensor_scalar(out=g2[:], in0=u[:], scalar1=0.5,
                                    scalar2=None, op0=OP.mult)
            nc.vector.scalar_tensor_tensor(out=u[:], in0=g1[:], scalar=1.0,
                                           in1=g2[:], op0=OP.add, op1=OP.mult)
            puT = ps_c("puT")
            for k in range(KH):
                nc.tensor.transpose(puT[:, k:k + 1],
                                    u[0:1, 128 * k:128 * (k + 1)], one1[:])
            uT = work.tile([128, KH], F32R, tag="uT", name="uT")
            nc.vector.tensor_copy(uT[:], puT[:, 0:KH])
            pmlA = ps_a("pmlA")
            pmlB = ps_a("pmlB")
            for k in range(KH):
                nc.tensor.matmul(pmlA[0:1, :], uT[:, k:k + 1],
                                 mlpw[:, E * k:E * k + 384],
                                 start=(k == 0), stop=(k == KH - 1))
                nc.tensor.matmul(pmlB[0:1, :], uT[:, k:k + 1],
                                 mlpw[:, E * k + 384:E * (k + 1)],
                                 start=(k == 0), stop=(k == KH - 1))
            pm = work.tile([1, E], F32, tag="pa", name="pm")
            nc.vector.tensor_tensor(out=pm[0:1, 0:384], in0=pmlA[0:1, :],
                                    in1=bias[0:1, 1536:1920], op=OP.add)
            nc.vector.tensor_tensor(out=pm[0:1, 384:768], in0=pmlB[0:1, :],
                                    in1=bias[0:1, 1920:2304], op=OP.add)
            sync_residual(pm)

        # ---- final LN + lm_head ----
        hfn, _ = layernorm(hP, lnfp[:, 0:KE], lnfp[:, KE:2 * KE], "f")
        phfA = ps_a("phfA")
        phfB = ps_a("phfB")
        for k in range(KE):
            tgt = phfA if k < 3 else phfB
            nc.tensor.transpose(tgt[0:1, 128 * (k % 3):128 * (k % 3 + 1)],
                                hfn[:, k:k + 1], ident[:])
        hfrow = work.tile([1, E], F32, tag="hrow", name="hfrow")
        nc.vector.tensor_copy(hfrow[0:1, 0:384], phfA[0:1, :])
        nc.vector.tensor_copy(hfrow[0:1, 384:768], phfB[0:1, :])
        hb = state.tile([128, E], F32)
        for i in range(2):
            phb = ps_b(f"phb{i}")
            nc.tensor.matmul(phb[:, 0:384], ones_row[:],
                             hfrow[0:1, 384 * i:384 * (i + 1)],
                             start=True, stop=True)
            nc.vector.tensor_copy(hb[:, 384 * i:384 * (i + 1)], phb[:, 0:384])
        for i in range(nlm):
            wt = lmw.tile([128, E], F32, tag="wt", name="wt")
            nc.sync.dma_start(wt[:], wte_d[i])
            pr = work.tile([128, E], F32, tag="lmprod", name="lmprod")
            nc.vector.tensor_tensor(out=pr[:], in0=wt[:], in1=hb[:], op=OP.mult)
            nc.vector.tensor_reduce(out=logits_sb[:, i:i + 1], in_=pr[:],
                                    axis=AX.X, op=OP.add)
        nc.sync.dma_start(logits_d[:], logits_sb[:])

    nc.compile()
    return nc


def shard_inputs(inputs, nlayers=L, nlm=NLM):
    """Build per-core in_maps from full inputs."""
    wte = np.asarray(inputs["wte"], np.float32)
    wpe = np.asarray(inputs["wpe"], np.float32)
    tok = int(np.asarray(inputs["input_ids"]).ravel()[0])
    x0 = wte[tok] + wpe[S]
    x0p = np.ascontiguousarray(x0.reshape(KE, 128).T)
    attn_w = np.asarray(inputs["attn_w"], np.float32)
    attn_b = np.asarray(inputs["attn_b"], np.float32)
    proj_w = np.asarray(inputs["proj_w"], np.float32)
    proj_b = np.asarray(inputs["proj_b"], np.float32)
    fc_w = np.asarray(inputs["fc_w"], np.float32)
    fc_b = np.asarray(inputs["fc_b"], np.float32)
    mlp_w = np.asarray(inputs["mlp_w"], np.float32)
    mlp_b = np.asarray(inputs["mlp_b"], np.float32)
    ln1_g = np.asarray(inputs["ln1_g"], np.float32)
    ln1_b = np.asarray(inputs["ln1_b"], np.float32)
    ln2_g = np.asarray(inputs["ln2_g"], np.float32)
    ln2_b = np.asarray(inputs["ln2_b"], np.float32)
    lnf_g = np.asarray(inputs["lnf_g"], np.float32)
    lnf_b = np.asarray(inputs["lnf_b"], np.float32)
    pk = np.asarray(inputs["past_key"], np.float32)
    pv = np.asarray(inputs["past_value"], np.float32)

    def pmaj(v):
        return np.ascontiguousarray(v.reshape(KE, 128).T)

    def pmajL(m):  # [nl, E] -> [nl, 128, KE]
        return np.ascontiguousarray(
            m[:nlayers].reshape(nlayers, KE, 128).transpose(0, 2, 1))

    lnf_pm = np.concatenate([pmaj(lnf_g), pmaj(lnf_b)], axis=1)
    ident = np.eye(128, dtype=np.float32)
    wte_pad = np.zeros((NCORES * VPC, E), np.float32)
    wte_pad[:V] = wte
    lnp_all = np.concatenate(
        [pmajL(ln1_g), pmajL(ln1_b), pmajL(ln2_g), pmajL(ln2_b)], axis=2)

    in_maps = []
    for c in range(NCORES):
        vhs = [2 * c, 2 * c + 1]
        qkvw = np.zeros((nlayers, E, 384), np.float32)
        qkvb = np.zeros((nlayers, 384), np.float32)
        projw = np.zeros((nlayers, 128, E), np.float32)
        kvc = np.zeros((nlayers, MAXS, ROW), np.float32)
        kvc[:, :, 192] = 1.0
        kvc[:, :, 257] = 1.0
        kvc[:, S, :] = 0.0
        for j, h in enumerate(vhs):
            if h >= H:
                continue
            for part, off in ((0, 0), (1, E), (2, 2 * E)):
                qkvw[:, :, 128 * part + 64 * j:128 * part + 64 * (j + 1)] = \
                    attn_w[:nlayers, :, off + 64 * h:off + 64 * (h + 1)]
                qkvb[:, 128 * part + 64 * j:128 * part + 64 * (j + 1)] = \
                    attn_b[:nlayers, off + 64 * h:off + 64 * (h + 1)]
            projw[:, 64 * j:64 * (j + 1), :] = \
                proj_w[:nlayers, 64 * h:64 * (h + 1), :]
            kvc[:, :S, 64 * j:64 * (j + 1)] = pk[:nlayers, 0, h, :S, :]
            kvc[:, :S, 128 + 65 * j:128 + 65 * j + 64] = pv[:nlayers, 0, h, :S, :]
        fcw = np.ascontiguousarray(fc_w[:nlayers, :, HID * c:HID * (c + 1)])
        fcb = fc_b[:nlayers, HID * c:HID * (c + 1)]
        mlpw = np.ascontiguousarray(mlp_w[:nlayers, HID * c:HID * (c + 1), :])
        bias = np.ascontiguousarray(np.concatenate(
            [qkvb, proj_b[:nlayers] / NCORES, fcb, mlp_b[:nlayers] / NCORES],
            axis=1))
        wtep = np.ascontiguousarray(
            wte_pad[c * VPC:(c + 1) * VPC].reshape(NLM, 128, E)[:nlm])
        in_maps.append({
            "x0p": x0p, "qkvw": qkvw, "fcw": fcw, "mlpw": mlpw,
            "projw": projw, "bias": bias, "lnp": lnp_all, "lnf": lnf_pm,
            "kvc": kvc, "wtep": wtep, "ident": ident,
        })
    return in_maps


def assemble(inputs, results, nlayers=L):
    logits_full = np.zeros((NCORES * VPC,), np.float32)
    for c in range(NCORES):
        lg = np.asarray(results[c]["logits"])
        n = lg.shape[1] * 128
        logits_full[c * VPC:c * VPC + n] = lg.T.reshape(-1)
    logits = logits_full[:V].reshape(1, 1, V)
    pk = np.array(np.asarray(inputs["past_key"], np.float32))
    pv = np.array(np.asarray(inputs["past_value"], np.float32))
    for c in range(NCORES):
        kn = np.asarray(results[c]["kvnew"])
        for j, h in enumerate([2 * c, 2 * c + 1]):
            if h >= H:
                continue
            pk[:nlayers, 0, h, S, :] = kn[:, 64 * j:64 * (j + 1)]
            pv[:nlayers, 0, h, S, :] = kn[:, 128 + 64 * j:128 + 64 * (j + 1)]
    return (logits, pk, pv)


_NC_CACHE = {}


def kernel(**inputs):
    if "nc" not in _NC_CACHE:
        _NC_CACHE["nc"] = build_nc()
    nc = _NC_CACHE["nc"]
    in_maps = shard_inputs(inputs)
    res = bass_utils.run_bass_kernel_spmd(nc, in_maps,
                                          core_ids=list(range(NCORES)))
    return assemble(inputs, res.results)


# revision 11
# speedup vs baseline: 1.1645x; 1.1645x over previous
"""GPT-2 single-token decode on 8 TRN2 NeuronCores (tensor-parallel).

Sharding: 16 virtual heads (12 real + 4 zero-pad), 2 per core; MLP hidden
3072 split 384/core; lm_head vocab split 6400/core (padded 51200).
Per-layer sync: AllGather of partial residual contributions + local
ones-matmul reduce.  Weight GEMVs run on PE with x stationary (f32r);
QK scores + lm_head dots run on DVE (natural [out,in] layouts).
KV cache rows are interleaved per-core as [K0|K1|V0|1|V1|1]; the ones
columns let the PV matmul accumulate the softmax denominator for free.
The new-token k/v never enters the cache tiles - its score/value term is
handled inline on partition 0.  Softmax skips max-subtraction (scores
are O(1) for this model; exp is safe).
"""
import numpy as np
from contextlib import ExitStack

import concourse.bass as bass
import concourse.tile as tile
from concourse import bacc, mybir
from concourse import bass_utils

dt = mybir.dt
F32 = dt.float32
F32R = dt.float32r
AF = mybir.ActivationFunctionType
OP = mybir.AluOpType
AX = mybir.AxisListType

NCORES = 8
L, H, D, E, V, MAXS = 12, 12, 64, 768, 50257, 1024
S = 1023            # past length (static)
NT = 8              # seq tiles of 128
HID = 384           # mlp hidden per core
VPC = 6400          # vocab rows per core (padded); 50 tiles of 128
NLM = VPC // 128
KE = E // 128       # 6 k-tiles over E
KH = HID // 128     # 3 k-tiles over HID
ROW = 258           # kv row: K0|K1|V0|1|V1|1


def build_nc(nlayers=L, nlm=NLM):
    nc = bacc.Bacc("TRN2", target_bir_lowering=False, debug=False,
                   num_devices=NCORES)
    x0_d = nc.dram_tensor("x0p", [128, KE], F32, kind="ExternalInput")
    qkvw_d = nc.dram_tensor("qkvw", [nlayers, E, 384], F32R, kind="ExternalInput")
    fcw_d = nc.dram_tensor("fcw", [nlayers, E, HID], F32R, kind="ExternalInput")
    mlpw_d = nc.dram_tensor("mlpw", [nlayers, HID, E], F32R, kind="ExternalInput")
    projw_d = nc.dram_tensor("projw", [nlayers, 128, E], F32R, kind="ExternalInput")
    bias_d = nc.dram_tensor("bias", [nlayers, 2304], F32, kind="ExternalInput")
    lnp_d = nc.dram_tensor("lnp", [nlayers, 128, 4 * KE], F32, kind="ExternalInput")
    lnf_d = nc.dram_tensor("lnf", [128, 2 * KE], F32, kind="ExternalInput")
    kvc_d = nc.dram_tensor("kvc", [nlayers, MAXS, ROW], F32, kind="ExternalInput")
    wte_d = nc.dram_tensor("wtep", [nlm, 128, E], F32, kind="ExternalInput")
    id_d = nc.dram_tensor("ident", [128, 128], F32, kind="ExternalInput")
    logits_d = nc.dram_tensor("logits", [128, nlm], F32, kind="ExternalOutput")
    kvnew_d = nc.dram_tensor("kvnew", [nlayers, 256], F32, kind="ExternalOutput")

    with tile.TileContext(nc) as tc, ExitStack() as ctx:
        state = ctx.enter_context(tc.tile_pool(name="state", bufs=1))
        wpool = ctx.enter_context(tc.tile_pool(name="wpool", bufs=3))
        kvpool = ctx.enter_context(tc.tile_pool(name="kvpool", bufs=2))
        cpool = ctx.enter_context(tc.tile_pool(name="cpool", bufs=2))
        work = ctx.enter_context(tc.tile_pool(name="work", bufs=2))
        lmw = ctx.enter_context(tc.tile_pool(name="lmw", bufs=4))
        ps = ctx.enter_context(tc.tile_pool(name="ps", bufs=1, space="PSUM"))
        dram = ctx.enter_context(tc.tile_pool(name="dram", bufs=2, space="DRAM"))

        def ps_a(name):   # [1, <=384] rows, 4 shared banks
            return ps.tile([1, 384], F32, tag="psa", bufs=4, name=name)

        def ps_b(name):   # [128, <=384], 2 shared banks
            return ps.tile([128, 384], F32, tag="psb", bufs=2, name=name)

        def ps_c(name):   # [128, <=8], 2 shared banks
            return ps.tile([128, 8], F32, tag="psc", bufs=2, name=name)

        # persistent state
        hP = state.tile([128, KE], F32)
        nc.sync.dma_start(hP[:], x0_d[:])
        ones_col = state.tile([128, 1], F32)
        nc.vector.memset(ones_col[:], 1.0)
        ones_row = state.tile([1, 128], F32)
        nc.vector.memset(ones_row[:], 1.0)
        ones8 = state.tile([NCORES, 1], F32)
        nc.vector.memset(ones8[:], 1.0)
        one1 = state.tile([1, 1], F32)
        nc.vector.memset(one1[:], 1.0)
        ident = state.tile([128, 128], F32)
        nc.sync.dma_start(ident[:], id_d[:])
        epsT = state.tile([1, 1], F32)
        nc.vector.memset(epsT[:], 1e-5)
        sq2piT = state.tile([128, 1], F32)
        nc.vector.memset(sq2piT[:], 0.7978845608028654)
        lnfp = state.tile([128, 2 * KE], F32)
        nc.sync.dma_start(lnfp[:], lnf_d[:])
        logits_sb = state.tile([128, nlm], F32)

        def layernorm(src, g_ap, b_ap, tag):
            """src [128,KE] F32 -> (xn F32, xr F32R), LN over all 768."""
            sq = work.tile([128, KE], F32, tag="sq", name="sq")
            nc.vector.tensor_tensor(out=sq[:], in0=src[:], in1=src[:], op=OP.mult)
            st = ps_a("st")
            nc.tensor.matmul(st[0:1, 0:KE], ones_col[:], src[:],
                             start=True, stop=True)
            nc.tensor.matmul(st[0:1, KE:2 * KE], ones_col[:], sq[:],
                             start=True, stop=True)
            mvr = work.tile([1, 4], F32, tag="mvr", name="mvr")
            nc.vector.tensor_reduce(out=mvr[0:1, 0:1], in_=st[0:1, 0:KE],
                                    axis=AX.X, op=OP.add)
            nc.vector.tensor_reduce(out=mvr[0:1, 3:4], in_=st[0:1, KE:2 * KE],
                                    axis=AX.X, op=OP.add)
            nc.scalar.mul(mvr[0:1, 0:1], mvr[0:1, 0:1], 1.0 / E)
            nc.scalar.activation(mvr[0:1, 1:2], mvr[0:1, 0:1], AF.Square)
            nc.vector.scalar_tensor_tensor(
                out=mvr[0:1, 2:3], in0=mvr[0:1, 3:4], scalar=1.0 / E,
                in1=mvr[0:1, 1:2], op0=OP.mult, op1=OP.subtract)
            nc.scalar.activation(mvr[0:1, 1:2], mvr[0:1, 2:3], AF.Sqrt,
                                 bias=epsT[0:1, 0:1])
            nc.vector.reciprocal(mvr[0:1, 1:2], mvr[0:1, 1:2])
            mb = ps_c("mb")
            nc.tensor.matmul(mb[:, 0:2], ones_row[:], mvr[0:1, 0:2],
                             start=True, stop=True)
            xc = work.tile([128, KE], F32, tag="xc", name="xc")
            nc.vector.tensor_scalar(out=xc[:], in0=src[:], scalar1=mb[:, 0:1],
                                    scalar2=None, op0=OP.subtract)
            xn = work.tile([128, KE], F32, tag=f"xn_{tag}", name=f"xn_{tag}")
            nc.vector.scalar_tensor_tensor(out=xn[:], in0=xc[:], scalar=mb[:, 1:2],
                                           in1=g_ap, op0=OP.mult, op1=OP.mult)
            nc.vector.tensor_tensor(out=xn[:], in0=xn[:], in1=b_ap, op=OP.add)
            xr = work.tile([128, KE], F32R, tag=f"xr_{tag}", name=f"xr_{tag}")
            nc.vector.tensor_copy(xr[:], xn[:])
            return xn, xr

        def sync_residual(partial_sb):
            """AllGather partial [1,E] from all cores, reduce, add into hP."""
            agin = dram.tile([1, E], F32, tag="agin", name="agin")
            agout = dram.tile([NCORES, E], F32, addr_space="Shared",
                              tag="agout", name="agout")
            nc.sync.dma_start(agin[:], partial_sb[:])
            nc.gpsimd.collective_compute(
                "AllGather", OP.bypass,
                replica_groups=[list(range(NCORES))],
                ins=[agin.opt()], outs=[agout.opt()])
            gath = work.tile([NCORES, E], F32, tag="gath", name="gath")
            nc.sync.dma_start(gath[:], agout[:])
            # sum the 8 partials straight into P-major: for each k-tile,
            # out[:, k] = gath[:, 128k:128k+128].T @ ones8
            hT = ps_c("hT")
            for k in range(KE):
                nc.tensor.matmul(hT[:, k:k + 1],
                                 gath[:, 128 * k:128 * (k + 1)], ones8[:],
                                 start=True, stop=True)
            nc.vector.tensor_tensor(out=hP[:], in0=hP[:], in1=hT[:, 0:KE],
                                    op=OP.add)

        for l in range(nlayers):
            # ---- per-layer DMAs ----
            qkvw = wpool.tile([128, KE * 384], F32R, tag="qkvw", name="qkvw")
            nc.sync.dma_start(
                qkvw[:].rearrange("p (k n) -> p k n", k=KE),
                qkvw_d[l].rearrange("(k p) n -> p k n", p=128))
            fcw = wpool.tile([128, KE * HID], F32R, tag="fcw", name="fcw")
            nc.sync.dma_start(
                fcw[:].rearrange("p (k n) -> p k n", k=KE),
                fcw_d[l].rearrange("(k p) n -> p k n", p=128))
            mlpw = wpool.tile([128, KH * E], F32R, tag="mlpw", name="mlpw")
            nc.sync.dma_start(
                mlpw[:].rearrange("p (k n) -> p k n", k=KH),
                mlpw_d[l].rearrange("(k p) n -> p k n", p=128))
            projw = wpool.tile([128, E], F32R, tag="projw", name="projw")
            nc.sync.dma_start(projw[:], projw_d[l])
            bias = cpool.tile([1, 2304], F32, tag="bias", name="bias")
            nc.sync.dma_start(bias[:], bias_d[l:l + 1, :])
            lnp = cpool.tile([128, 4 * KE], F32, tag="lnp", name="lnp")
            nc.sync.dma_start(lnp[:], lnp_d[l])
            kv = kvpool.tile([128, NT * ROW], F32, tag="kv", name="kv")
            nc.sync.dma_start(
                kv[:].rearrange("p (t n) -> p t n", t=NT),
                kvc_d[l].rearrange("(t p) n -> p t n", p=128))

            # ---- LN1 + qkv ----
            xn, xr = layernorm(hP, lnp[:, 0:KE], lnp[:, KE:2 * KE], "a")
            pqkv = ps_a("pqkv")
            for k in range(KE):
                nc.tensor.matmul(pqkv[0:1, :], xr[:, k:k + 1],
                                 qkvw[:, 384 * k:384 * (k + 1)],
                                 start=(k == 0), stop=(k == KE - 1))
            qkv = work.tile([1, 384], F32, tag="qkv", name="qkv")
            nc.vector.tensor_tensor(out=qkv[:], in0=pqkv[0:1, :],
                                    in1=bias[0:1, 0:384], op=OP.add)
            nc.sync.dma_start(kvnew_d[l:l + 1, :], qkv[0:1, 128:384])

            # ---- attention ----
            qb = ps_b("qb")
            nc.tensor.matmul(qb[:, 0:128], ones_row[:], qkv[0:1, 0:128],
                             start=True, stop=True)
            qq = work.tile([128, 128], F32, tag="qq", name="qq")
            nc.vector.tensor_copy(qq[:], qb[:, 0:128])

            scores = work.tile([128, 16], F32, tag="scores", name="scores")
            nc.vector.memset(scores[:], 0.0)
            for t in range(NT):
                P = 127 if t == NT - 1 else 128
                prod = work.tile([128, 128], F32, tag="prod", name="prod",
                                 bufs=4)
                nc.vector.tensor_tensor(out=prod[0:P, :],
                                        in0=kv[0:P, ROW * t:ROW * t + 128],
                                        in1=qq[0:P, :], op=OP.mult)
                ascr = work.tile([128, 64], F32, tag="ascr", name="ascr",
                                 bufs=4)
                nc.scalar.activation(ascr[0:P, :], prod[0:P, 0:64], AF.Copy,
                                     accum_out=scores[0:P, t:t + 1])
                ascr2 = work.tile([128, 64], F32, tag="ascr", name="ascr2",
                                  bufs=4)
                nc.scalar.activation(ascr2[0:P, :], prod[0:P, 64:128], AF.Copy,
                                     accum_out=scores[0:P, 8 + t:9 + t])
            pexp = work.tile([128, 16], F32, tag="pexp", name="pexp")
            nc.scalar.activation(pexp[:], scores[:], AF.Exp, scale=0.125)
            # new-token score, partition 0
            pn = work.tile([1, 130], F32, tag="pn", name="pn")
            nc.vector.tensor_tensor(out=pn[0:1, 2:130], in0=qkv[0:1, 0:128],
                                    in1=qkv[0:1, 128:256], op=OP.mult)
            nc.vector.tensor_reduce(out=pn[0:1, 0:1], in_=pn[0:1, 2:66],
                                    axis=AX.X, op=OP.add)
            nc.vector.tensor_reduce(out=pn[0:1, 1:2], in_=pn[0:1, 66:130],
                                    axis=AX.X, op=OP.add)
            nc.scalar.activation(pn[0:1, 0:2], pn[0:1, 0:2], AF.Exp, scale=0.125)

            po = [ps_a("po0"), ps_a("po1")]
            for h in range(2):
                for t in range(NT):
                    P = 127 if t == NT - 1 else 128
                    base = ROW * t + 128 + 65 * h
                    nc.tensor.matmul(po[h][0:1, 0:65],
                                     pexp[0:P, 8 * h + t:8 * h + t + 1],
                                     kv[0:P, base:base + 65],
                                     start=(t == 0), stop=(t == NT - 1))
            onrm = work.tile([1, 128], F32, tag="onrm", name="onrm")
            z2 = work.tile([1, 2], F32, tag="z2", name="z2")
            for h in range(2):
                nc.vector.scalar_tensor_tensor(
                    out=onrm[0:1, 64 * h:64 * (h + 1)],
                    in0=qkv[0:1, 256 + 64 * h:256 + 64 * (h + 1)],
                    scalar=pn[0:1, h:h + 1], in1=po[h][0:1, 0:64],
                    op0=OP.mult, op1=OP.add)
                nc.vector.tensor_tensor(out=z2[0:1, h:h + 1],
                                        in0=po[h][0:1, 64:65],
                                        in1=pn[0:1, h:h + 1], op=OP.add)
            nc.vector.reciprocal(z2[:], z2[:])
            for h in range(2):
                nc.vector.tensor_scalar(out=onrm[0:1, 64 * h:64 * (h + 1)],
                                        in0=onrm[0:1, 64 * h:64 * (h + 1)],
                                        scalar1=z2[0:1, h:h + 1], scalar2=None,
                                        op0=OP.mult)
            poT = ps_c("poT")
            nc.tensor.transpose(poT[:, 0:1], onrm[:], one1[:])
            oT = work.tile([128, 1], F32R, tag="oT", name="oT")
            nc.vector.tensor_copy(oT[:], poT[:, 0:1])
            pprA = ps_a("pprA")
            pprB = ps_a("pprB")
            nc.tensor.matmul(pprA[0:1, :], oT[:], projw[:, 0:384],
                             start=True, stop=True)
            nc.tensor.matmul(pprB[0:1, :], oT[:], projw[:, 384:768],
                             start=True, stop=True)
            pa = work.tile([1, E], F32, tag="pa", name="pa")
            nc.vector.tensor_tensor(out=pa[0:1, 0:384], in0=pprA[0:1, :],
                                    in1=bias[0:1, 384:768], op=OP.add)
            nc.vector.tensor_tensor(out=pa[0:1, 384:768], in0=pprB[0:1, :],
                                    in1=bias[0:1, 768:1152], op=OP.add)
            sync_residual(pa)

            # ---- LN2 + MLP ----
            x2n, x2r = layernorm(hP, lnp[:, 2 * KE:3 * KE], lnp[:, 3 * KE:4 * KE],
                                 "m")
            pfc = ps_a("pfc")
            for k in range(KE):
                nc.tensor.matmul(pfc[0:1, :], x2r[:, k:k + 1],
                                 fcw[:, HID * k:HID * (k + 1)],
                                 start=(k == 0), stop=(k == KE - 1))
            u = work.tile([1, HID], F32, tag="u", name="u")
            nc.vector.tensor_tensor(out=u[:], in0=pfc[0:1, :],
                                    in1=bias[0:1, 1152:1536], op=OP.add)
            puT = ps_c("puT")
            for k in range(KH):
                nc.tensor.transpose(puT[:, k:k + 1],
                                    u[0:1, 128 * k:128 * (k + 1)], one1[:])
            # tanh-approx gelu in P-major [128,3]:
            # 0.5*x*(1+tanh(c*(x+0.044715 x^3)))
            ub = work.tile([128, KH], F32, tag="ub", name="ub")
            nc.vector.tensor_copy(ub[:], puT[:, 0:KH])
            g1 = work.tile([128, KH], F32, tag="g1", name="g1")
            nc.vector.tensor_tensor(out=g1[:], in0=ub[:], in1=ub[:], op=OP.mult)
            nc.vector.tensor_tensor(out=g1[:], in0=g1[:], in1=ub[:], op=OP.mult)
            nc.vector.scalar_tensor_tensor(out=g1[:], in0=g1[:], scalar=0.044715,
                                           in1=ub[:], op0=OP.mult, op1=OP.add)
            nc.scalar.activation(g1[:], g1[:], AF.Tanh, scale=sq2piT[:])
            g2 = work.tile([128, KH], F32, tag="g2", name="g2")
            nc.vector.tensor_scalar(out=g2[:], in0=ub[:], scalar1=0.5,
                                    scalar2=None, op0=OP.mult)
            uT = work.tile([128, KH], F32R, tag="uT", name="uT")
            nc.vector.scalar_tensor_tensor(out=uT[:], in0=g1[:], scalar=1.0,
                                           in1=g2[:], op0=OP.add, op1=OP.mult)
            pmlA = ps_a("pmlA")
            pmlB = ps_a("pmlB")
            for k in range(KH):
                nc.tensor.matmul(pmlA[0:1, :], uT[:, k:k + 1],
                                 mlpw[:, E * k:E * k + 384],
                                 start=(k == 0), stop=(k == KH - 1))
                nc.tensor.matmul(pmlB[0:1, :], uT[:, k:k + 1],
                                 mlpw[:, E * k + 384:E * (k + 1)],
                                 start=(k == 0), stop=(k == KH - 1))
            pm = work.tile([1, E], F32, tag="pa", name="pm")
            nc.vector.tensor_tensor(out=pm[0:1, 0:384], in0=pmlA[0:1, :],
                                    in1=bias[0:1, 1536:1920], op=OP.add)
            nc.vector.tensor_tensor(out=pm[0:1, 384:768], in0=pmlB[0:1, :],
                                    in1=bias[0:1, 1920:2304], op=OP.add)
            sync_residual(pm)

        # ---- final LN + lm_head ----
        hfn, _ = layernorm(hP, lnfp[:, 0:KE], lnfp[:, KE:2 * KE], "f")
        phfA = ps_a("phfA")
        phfB = ps_a("phfB")
        for k in range(KE):
            tgt = phfA if k < 3 else phfB
            nc.tensor.transpose(tgt[0:1, 128 * (k % 3):128 * (k % 3 + 1)],
                                hfn[:, k:k + 1], ident[:])
        hfrow = work.tile([1, E], F32, tag="hrow", name="hfrow")
        nc.vector.tensor_copy(hfrow[0:1, 0:384], phfA[0:1, :])
        nc.vector.tensor_copy(hfrow[0:1, 384:768], phfB[0:1, :])
        hb = state.tile([128, E], F32)
        for i in range(2):
            phb = ps_b(f"phb{i}")
            nc.tensor.matmul(phb[:, 0:384], ones_row[:],
                             hfrow[0:1, 384 * i:384 * (i + 1)],
                             start=True, stop=True)
            nc.vector.tensor_copy(hb[:, 384 * i:384 * (i + 1)], phb[:, 0:384])
        for i in range(nlm):
            wt = lmw.tile([128, E], F32, tag="wt", name="wt", bufs=4)
            nc.sync.dma_start(wt[:], wte_d[i])
            pr = work.tile([128, E], F32, tag="lmprod", name="lmprod", bufs=2)
            nc.vector.tensor_tensor(out=pr[:], in0=wt[:], in1=hb[:], op=OP.mult)
            lscr = work.tile([128, E], F32, tag="lmscr", name="lmscr", bufs=2)
            nc.scalar.activation(lscr[:], pr[:], AF.Copy,
                                 accum_out=logits_sb[:, i:i + 1])
        nc.sync.dma_start(logits_d[:], logits_sb[:])

    nc.compile()
    return nc


def shard_inputs(inputs, nlayers=L, nlm=NLM):
    """Build per-core in_maps from full inputs."""
    wte = np.asarray(inputs["wte"], np.float32)
    wpe = np.asarray(inputs["wpe"], np.float32)
    tok = int(np.asarray(inputs["input_ids"]).ravel()[0])
    x0 = wte[tok] + wpe[S]
    x0p = np.ascontiguousarray(x0.reshape(KE, 128).T)
    attn_w = np.asarray(inputs["attn_w"], np.float32)
    attn_b = np.asarray(inputs["attn_b"], np.float32)
    proj_w = np.asarray(inputs["proj_w"], np.float32)
    proj_b = np.asarray(inputs["proj_b"], np.float32)
    fc_w = np.asarray(inputs["fc_w"], np.float32)
    fc_b = np.asarray(inputs["fc_b"], np.float32)
    mlp_w = np.asarray(inputs["mlp_w"], np.float32)
    mlp_b = np.asarray(inputs["mlp_b"], np.float32)
    ln1_g = np.asarray(inputs["ln1_g"], np.float32)
    ln1_b = np.asarray(inputs["ln1_b"], np.float32)
    ln2_g = np.asarray(inputs["ln2_g"], np.float32)
    ln2_b = np.asarray(inputs["ln2_b"], np.float32)
    lnf_g = np.asarray(inputs["lnf_g"], np.float32)
    lnf_b = np.asarray(inputs["lnf_b"], np.float32)
    pk = np.asarray(inputs["past_key"], np.float32)
    pv = np.asarray(inputs["past_value"], np.float32)

    def pmaj(v):
        return np.ascontiguousarray(v.reshape(KE, 128).T)

    def pmajL(m):  # [nl, E] -> [nl, 128, KE]
        return np.ascontiguousarray(
            m[:nlayers].reshape(nlayers, KE, 128).transpose(0, 2, 1))

    lnf_pm = np.concatenate([pmaj(lnf_g), pmaj(lnf_b)], axis=1)
    ident = np.eye(128, dtype=np.float32)
    wte_pad = np.zeros((NCORES * VPC, E), np.float32)
    wte_pad[:V] = wte
    lnp_all = np.concatenate(
        [pmajL(ln1_g), pmajL(ln1_b), pmajL(ln2_g), pmajL(ln2_b)], axis=2)

    in_maps = []
    for c in range(NCORES):
        vhs = [2 * c, 2 * c + 1]
        qkvw = np.zeros((nlayers, E, 384), np.float32)
        qkvb = np.zeros((nlayers, 384), np.float32)
        projw = np.zeros((nlayers, 128, E), np.float32)
        kvc = np.zeros((nlayers, MAXS, ROW), np.float32)
        kvc[:, :, 192] = 1.0
        kvc[:, :, 257] = 1.0
        kvc[:, S, :] = 0.0
        for j, h in enumerate(vhs):
            if h >= H:
                continue
            for part, off in ((0, 0), (1, E), (2, 2 * E)):
                qkvw[:, :, 128 * part + 64 * j:128 * part + 64 * (j + 1)] = \
                    attn_w[:nlayers, :, off + 64 * h:off + 64 * (h + 1)]
                qkvb[:, 128 * part + 64 * j:128 * part + 64 * (j + 1)] = \
                    attn_b[:nlayers, off + 64 * h:off + 64 * (h + 1)]
            projw[:, 64 * j:64 * (j + 1), :] = \
                proj_w[:nlayers, 64 * h:64 * (h + 1), :]
            kvc[:, :S, 64 * j:64 * (j + 1)] = pk[:nlayers, 0, h, :S, :]
            kvc[:, :S, 128 + 65 * j:128 + 65 * j + 64] = pv[:nlayers, 0, h, :S, :]
        fcw = np.ascontiguousarray(fc_w[:nlayers, :, HID * c:HID * (c + 1)])
        fcb = fc_b[:nlayers, HID * c:HID * (c + 1)]
        mlpw = np.ascontiguousarray(mlp_w[:nlayers, HID * c:HID * (c + 1), :])
        bias = np.ascontiguousarray(np.concatenate(
            [qkvb, proj_b[:nlayers] / NCORES, fcb, mlp_b[:nlayers] / NCORES],
            axis=1))
        wtep = np.ascontiguousarray(
            wte_pad[c * VPC:(c + 1) * VPC].reshape(NLM, 128, E)[:nlm])
        in_maps.append({
            "x0p": x0p, "qkvw": qkvw, "fcw": fcw, "mlpw": mlpw,
            "projw": projw, "bias": bias, "lnp": lnp_all, "lnf": lnf_pm,
            "kvc": kvc, "wtep": wtep, "ident": ident,
        })
    return in_maps


def assemble(inputs, results, nlayers=L):
    logits_full = np.zeros((NCORES * VPC,), np.float32)
    for c in range(NCORES):
        lg = np.asarray(results[c]["logits"])
        n = lg.shape[1] * 128
        logits_full[c * VPC:c * VPC + n] = lg.T.reshape(-1)
    logits = logits_full[:V].reshape(1, 1, V)
    pk = np.array(np.asarray(inputs["past_key"], np.float32))
    pv = np.array(np.asarray(inputs["past_value"], np.float32))
    for c in range(NCORES):
        kn = np.asarray(results[c]["kvnew"])
        for j, h in enumerate([2 * c, 2 * c + 1]):
            if h >= H:
                continue
            pk[:nlayers, 0, h, S, :] = kn[:, 64 * j:64 * (j + 1)]
            pv[:nlayers, 0, h, S, :] = kn[:, 128 + 64 * j:128 + 64 * (j + 1)]
    return (logits, pk, pv)


_NC_CACHE = {}


def kernel(**inputs):
    if "nc" not in _NC_CACHE:
        _NC_CACHE["nc"] = build_nc()
    nc = _NC_CACHE["nc"]
    in_maps = shard_inputs(inputs)
    res = bass_utils.run_bass_kernel_spmd(nc, in_maps,
                                          core_ids=list(range(NCORES)))
    return assemble(inputs, res.results)
